# revision 83
# baseline (speedup 1.0000x reference)
import sys

if "/opt/trn_rl_repo" not in sys.path:
    sys.path.insert(0, "/opt/trn_rl_repo")

import numpy as np

import concourse.bass as bass
import concourse.bacc as bacc
import concourse.mybir as mybir
from concourse.masks import make_identity
from concourse.tile import TileContext

# Problem dims (hardcoded per contract)
B, CIN, COUT, F, N, K = 128, 16, 16, 512, 32, 2
NCORES = 8
BS = B // NCORES          # batch shard per core = 16
P = 128                   # partitions
FL = 4                    # f = fh*FL + fl, fh in [0,128), fl in [0,4)
NG = 4                    # node groups
GN = N // NG              # 8 nodes per group
U = GN * K                # 16 m-positions per group (u = 2*nl + k)
CM = COUT * N * K         # 1024 (c, m) columns
WN = COUT * U             # 256 matmul output columns per group

_nc_cache = None


def _build():
    """out[b,c,f,2n+k] = sum_i x[b,i,f,n]*Weff[n,i,c,k] + beff[n,c]  per core.

    Layout: f is split as (fh, fl) with fh on SBUF partitions so every DMA
    runs at fat-descriptor line rate:
      - host pre-repacks x to bf16 xq[fh, (fl, g, nl, i)] -> one 512KB DMA
        per b with 4KB contiguous per-partition runs
      - PE-transpose -> XT_g[(nl,i), (fl, fh)] bf16; psum drained by
        DVE (2x bf16) and ACT copies
      - bf16 matmul lhsT=XT rhs=block-diag bigw -> psum f32 [fh, (c,u)];
        bias lands via a K=1 ones x bias-row matmul on even fl (ACT copy
        evac) and via DVE tensor_add on odd fl, balancing PE/ACT/DVE
      - software pipeline: transposes of b+1 issue before matmuls of b so
        PE's in-order queue never starves the evac engines
      - one 1MB bf16 store per b with (fl,m)=512B contiguous DRAM runs;
        host upcasts the bf16 output to f32 (output tolerance is 2e-2)
    """
    nc = bacc.Bacc()
    f32 = mybir.dt.float32
    bf16 = mybir.dt.bfloat16
    # x ships pre-repacked from host, col = fl*512+g*128+nl*16+i of
    # x[b, i, fh*4+fl, g*8+nl]. The fl 0-1 half is int8 with per-(b,fh)
    # scales (dequantized on the otherwise-idle Pool engine), fl 2-3 stays
    # bf16 — halves half the input traffic at ~0.6% total error vs 2e-2
    i8 = mybir.dt.int8
    HC = CIN * FL * N // 2
    xq8 = nc.declare_dram_parameter("xq8", [BS, P, HC], i8, isOutput=False)
    xqb = nc.declare_dram_parameter("xqb", [BS, P, HC], bf16, isOutput=False)
    xs = nc.declare_dram_parameter("xs", [P, BS], f32, isOutput=False)
    # block-diagonal weights, rows nl*16+i, shipped bf16 (on-chip expansion
    # of a compact form is blocked by the 32-partition alignment rule)
    bigw = nc.declare_dram_parameter("bigw", [P, NG * WN], bf16, isOutput=False)
    bias = nc.declare_dram_parameter("bias", [CM], f32, isOutput=False)
    # output ships as bf16 (tolerance is 2e-2; host upcasts) — halves the
    # store traffic, which dominates the DMA roofline
    out = nc.declare_dram_parameter("out", [BS, COUT, F, N * K], bf16, isOutput=True)

    with TileContext(nc) as tc:
        with (
            tc.tile_pool(name="const", bufs=1) as const,
            tc.tile_pool(name="xin", bufs=4) as xpool,
            tc.tile_pool(name="xt", bufs=8) as xtpool,
            tc.tile_pool(name="deq", bufs=3) as deqpool,
            tc.tile_pool(name="stage", bufs=8) as stpool,
            tc.tile_pool(name="pt", bufs=2, space="PSUM") as ptpool,
            tc.tile_pool(name="pm", bufs=3, space="PSUM") as pmpool,
        ):
            # bias row + weights via Pool/SWDGE before make_identity: the
            # 1-descriptor brow lands ~1.75us in (bias chain starts early),
            # the HWDGE slot frees up so L0 leads SP's queue
            brow = const.tile([1, CM], f32, tag="brow")
            nc.gpsimd.dma_start(out=brow[:], in_=bias[None, :])
            xst = const.tile([P, BS], f32, tag="xst")
            nc.gpsimd.dma_start(out=xst[:], in_=xs[:, :])
            wt = const.tile([P, NG * WN], bf16, tag="wtr")
            nc.gpsimd.dma_start(out=wt[:], in_=bigw[:, :])

            ident0 = const.tile([P, P], f32)
            make_identity(nc, ident0)
            # bf16 identity: ScalarE cast-copy, also puts transposes' dep on
            # a single engine
            ident = const.tile([P, P], bf16, tag="ident2")
            nc.scalar.copy(out=ident[:], in_=ident0[:])
            identr = ident[:, :]

            xins = []

            def load(b):
                # deep rings: loads prefetch far ahead so the DMA engines
                # stay busy while late-pipeline stores trail compute
                x8 = xpool.tile([P, HC], i8, bufs=10)
                nc.sync.dma_start(out=x8[:], in_=xq8[b])
                xb = xpool.tile([P, HC], bf16, tag="xbf", bufs=10)
                nc.sync.dma_start(out=xb[:], in_=xqb[b])
                xins.append((x8, xb))

            browr = const.tile([1, CM], bf16, tag="browr")
            nc.scalar.copy(out=browr[:], in_=brow[:])
            ones0 = const.tile([1, P], f32, tag="ones0")
            nc.gpsimd.memset(ones0, 1.0)
            onesr = const.tile([1, P], bf16, tag="onesr")
            nc.scalar.copy(out=onesr[:], in_=ones0[:])
            # broadcast bias tile for the DVE-add half of the evacs
            bt = const.tile([P, CM], f32)
            for h in range(2):
                pb = ptpool.tile([P, CM // 2], f32, tag="pt")
                nc.tensor.matmul(
                    pb[:],
                    onesr[:],
                    browr[:, h * (CM // 2) : (h + 1) * (CM // 2)],
                    start=True,
                    stop=True,
                )
                nc.scalar.copy(
                    out=bt[:, h * (CM // 2) : (h + 1) * (CM // 2)], in_=pb[:]
                )
            # bias cols (g, c, u) -> view (p, g, c, u)
            btv = bt[:, :].rearrange("p (g c u) -> p g c u", g=NG, c=COUT)

            load(0)
            load(1)
            load(2)
            load(3)
            load(4)
            load(5)
            load(6)
            load(7)

            def transpose_phase(b):
                x8, xb = xins[b]
                # dequant fl 0-1 on Pool: bf16 = int8 * scale[b,fh]
                deq = deqpool.tile([P, HC], bf16)
                nc.gpsimd.tensor_scalar(
                    out=deq[:],
                    in0=x8[:],
                    scalar1=xst[:, b : b + 1],
                    scalar2=None,
                    op0=mybir.AluOpType.mult,
                )
                # ---- transpose: XT_g[(nl*16+i), fl*128 + fh]
                xts = []
                for g in range(NG):
                    pt = ptpool.tile([P, FL * P], bf16)
                    for fl in range(FL):
                        src = (
                            deq[:, fl * 512 + g * P : fl * 512 + (g + 1) * P]
                            if fl < 2
                            else xb[:, (fl - 2) * 512 + g * P : (fl - 2) * 512 + (g + 1) * P]
                        )
                        nc.tensor.transpose(
                            pt[:, fl * P : (fl + 1) * P],
                            src,
                            identr,
                        )
                    xt = xtpool.tile([P, FL * P], bf16)
                    # DVE all-bf16 copies run in 2x mode; split with ACT
                    if g % 2 == 0:
                        nc.vector.tensor_copy(out=xt[:], in_=pt[:])
                    else:
                        nc.scalar.copy(out=xt[:], in_=pt[:])
                    xts.append(xt)
                return xts

            def matmul_store_phase(b, xts):
                # ---- bf16 matmul (+bias row) -> copy evac into STB
                stb = stpool.tile([P, COUT * FL * N * K], bf16)
                stv = stb[:, :].rearrange(
                    "p (c fl m) -> p c fl m", c=COUT, fl=FL
                )
                for fl in range(FL):
                    # one 1024-col psum tile (2 banks) per fl: 4 group
                    # matmuls, then a single fat evac (fewer engine-init
                    # penalties than per-group-pair evacs)
                    pm = pmpool.tile([P, NG * WN], f32)
                    pe_bias = fl % 2 == 0
                    for g in range(NG):
                        nc.tensor.matmul(
                            pm[:, g * WN : (g + 1) * WN],
                            xts[g][:, fl * P : (fl + 1) * P],
                            wt[:, g * WN : (g + 1) * WN],
                            start=True,
                            stop=not pe_bias,
                        )
                        if pe_bias:
                            # bias via K=1 accumulate: psum += ones.T @ brow
                            nc.tensor.matmul(
                                pm[:, g * WN : (g + 1) * WN],
                                onesr[:],
                                browr[:, g * WN : (g + 1) * WN],
                                start=False,
                                stop=True,
                            )
                    # dst cols c*256 + fl*64 + g*U + u
                    dst = stv[:, :, fl, :].rearrange("p c (g u) -> p g c u", g=NG)
                    src = pm[:, :].rearrange("p (g c u) -> p g c u", g=NG, c=COUT)
                    # PE-bias fls evac as ACT copy; the others as DVE-add
                    if pe_bias:
                        nc.scalar.copy(out=dst, in_=src)
                    else:
                        nc.vector.tensor_add(out=dst, in0=src, in1=btv[:, :])

                # ---- store: out[b, c, fh*4+fl, m] <- stb[fh, (c, fl, m)]
                # bf16: full store per b — (fl m) runs are 512B at full
                # descriptor rate; an fl-half split would drop to 256B (2x)
                nc.sync.dma_start(
                    out=out[b].rearrange("c (fh fl) m -> fh c fl m", fl=FL),
                    in_=stb[:, :].rearrange("p (c fl m) -> p c fl m", c=COUT, fl=FL),
                )

            # software pipeline: transposes/xt-copies for b+1 issue BEFORE
            # the matmul+evac phase of b, so PE's in-order queue never
            # starves ACT/DVE of next-batch XT work during b's matmuls
            pend = transpose_phase(0)
            for b in range(BS):
                if b + 8 < BS:
                    load(b + 8)
                if b + 1 < BS:
                    nxt = transpose_phase(b + 1)
                else:
                    nxt = None
                matmul_store_phase(b, pend)
                pend = nxt
    nc.compile()
    return nc


def _fold_weights(W1, b1, W2, b2):
    # Weff[n,i,c,k] = sum_o W1[n,i,o,k] * W2[n,o,c]; beff[n,c] = b1[n]@W2[n] + b2[n]
    Weff = np.einsum("niok,noc->nick", W1, W2).astype(np.float32)
    beff = (np.einsum("no,noc->nc", b1, W2) + b2).astype(np.float32)

    # bigw[nl*CIN + i, g*WN + c*U + 2*nl + k] = Weff[g*GN + nl, i, c, k]
    bigw = np.zeros((GN, CIN, NG, COUT, U), np.float32)  # [nl, i, g, c, u]
    nn, ii, gg, cc, kk = np.meshgrid(
        np.arange(GN), np.arange(CIN), np.arange(NG), np.arange(COUT),
        np.arange(K), indexing="ij",
    )
    w5 = Weff.reshape(NG, GN, CIN, COUT, K)  # [g, nl, i, c, k]
    bigw[nn, ii, gg, cc, 2 * nn + kk] = w5[gg, nn, ii, cc, kk]
    import ml_dtypes

    bigw = np.ascontiguousarray(bigw.reshape(P, NG * WN).astype(ml_dtypes.bfloat16))

    # bias_flat[g*WN + c*U + u] = beff[g*GN + u//K, c]  (matmul rhs layout)
    b4 = beff.reshape(NG, GN, COUT)  # [g, nl, c]
    bias_gcu = np.repeat(b4, K, axis=1)  # [g, u=(nl,k), c]
    bias_flat = np.ascontiguousarray(bias_gcu.transpose(0, 2, 1).reshape(-1))
    return bigw, bias_flat


def kernel(x, W1, b1, W2, b2):
    global _nc_cache
    import ml_dtypes
    from concourse.bass_utils import run_bass_kernel_spmd

    # host pre-repack: xq[b, fh, fl*512+g*128+nl*16+i]; fl 0-1 half as
    # int8 + per-(b,fh) scale, fl 2-3 half as bf16
    x = np.asarray(x, dtype=np.float32)
    xb2 = (
        x.reshape(B, CIN, P, FL, NG, GN)
        .transpose(0, 2, 3, 4, 5, 1)
        .reshape(B, P, CIN * FL * N)
    )
    HC = CIN * FL * N // 2
    lo, hi = xb2[:, :, :HC], xb2[:, :, HC:]
    sc = np.maximum(np.abs(lo).max(axis=2) / 127.0, 1e-30)  # [B, P]
    xq8 = np.clip(np.rint(lo / sc[:, :, None]), -127, 127).astype(np.int8)
    xqb = np.ascontiguousarray(hi.astype(ml_dtypes.bfloat16))
    xq8 = np.ascontiguousarray(xq8)
    bigw, bias_flat = _fold_weights(
        np.asarray(W1, np.float32),
        np.asarray(b1, np.float32),
        np.asarray(W2, np.float32),
        np.asarray(b2, np.float32),
    )
    if _nc_cache is None:
        _nc_cache = _build()
    nc = _nc_cache
    in_maps = [
        {
            "xq8": xq8[d * BS : (d + 1) * BS],
            "xqb": xqb[d * BS : (d + 1) * BS],
            "xs": np.ascontiguousarray(sc[d * BS : (d + 1) * BS].T),
            "bigw": bigw,
            "bias": bias_flat,
        }
        for d in range(NCORES)
    ]
    res = run_bass_kernel_spmd(nc, in_maps, list(range(NCORES)))
    return np.concatenate(
        [res.results[d]["out"].astype(np.float32) for d in range(NCORES)], axis=0
    )


# revision 87
# speedup vs baseline: 1.0034x; 1.0034x over previous
import sys

if "/opt/trn_rl_repo" not in sys.path:
    sys.path.insert(0, "/opt/trn_rl_repo")

import numpy as np

import concourse.bass as bass
import concourse.bacc as bacc
import concourse.mybir as mybir
from concourse.masks import make_identity
from concourse.tile import TileContext

# Problem dims (hardcoded per contract)
B, CIN, COUT, F, N, K = 128, 16, 16, 512, 32, 2
NCORES = 8
BS = B // NCORES          # batch shard per core = 16
P = 128                   # partitions
FL = 4                    # f = fh*FL + fl, fh in [0,128), fl in [0,4)
NG = 4                    # node groups
GN = N // NG              # 8 nodes per group
U = GN * K                # 16 m-positions per group (u = 2*nl + k)
CM = COUT * N * K         # 1024 (c, m) columns
WN = COUT * U             # 256 matmul output columns per group

_nc_cache = None


def _build():
    """out[b,c,f,2n+k] = sum_i x[b,i,f,n]*Weff[n,i,c,k] + beff[n,c]  per core.

    Layout: f is split as (fh, fl) with fh on SBUF partitions so every DMA
    runs at fat-descriptor line rate:
      - host pre-repacks x to bf16 xq[fh, (fl, g, nl, i)] -> one 512KB DMA
        per b with 4KB contiguous per-partition runs
      - PE-transpose -> XT_g[(nl,i), (fl, fh)] bf16; psum drained by
        DVE (2x bf16) and ACT copies
      - bf16 matmul lhsT=XT rhs=block-diag bigw -> psum f32 [fh, (c,u)];
        bias lands via a K=1 ones x bias-row matmul on even fl (ACT copy
        evac) and via DVE tensor_add on odd fl, balancing PE/ACT/DVE
      - software pipeline: transposes of b+1 issue before matmuls of b so
        PE's in-order queue never starves the evac engines
      - one 1MB bf16 store per b with (fl,m)=512B contiguous DRAM runs;
        host upcasts the bf16 output to f32 (output tolerance is 2e-2)
    """
    nc = bacc.Bacc()
    f32 = mybir.dt.float32
    bf16 = mybir.dt.bfloat16
    # x ships pre-repacked from host, col = fl*512+g*128+nl*16+i of
    # x[b, i, fh*4+fl, g*8+nl]. The fl 0-1 half is int8 with per-(b,fh)
    # scales (dequantized on the otherwise-idle Pool engine), fl 2-3 stays
    # bf16 — halves half the input traffic at ~0.6% total error vs 2e-2
    i8 = mybir.dt.int8
    HC = CIN * FL * N // 2
    xq8 = nc.declare_dram_parameter("xq8", [BS, P, HC], i8, isOutput=False)
    xqb = nc.declare_dram_parameter("xqb", [BS, P, HC], bf16, isOutput=False)
    xs = nc.declare_dram_parameter("xs", [P, BS], f32, isOutput=False)
    # block-diagonal weights, rows nl*16+i, shipped bf16 (on-chip expansion
    # of a compact form is blocked by the 32-partition alignment rule)
    bigw = nc.declare_dram_parameter("bigw", [P, NG * WN], bf16, isOutput=False)
    bias = nc.declare_dram_parameter("bias", [CM], f32, isOutput=False)
    # output ships as bf16 (tolerance is 2e-2; host upcasts) — halves the
    # store traffic, which dominates the DMA roofline
    out = nc.declare_dram_parameter("out", [BS, COUT, F, N * K], bf16, isOutput=True)

    with TileContext(nc) as tc:
        with (
            tc.tile_pool(name="const", bufs=1) as const,
            tc.tile_pool(name="xin", bufs=4) as xpool,
            tc.tile_pool(name="xt", bufs=8) as xtpool,
            tc.tile_pool(name="deq", bufs=3) as deqpool,
            tc.tile_pool(name="stage", bufs=8) as stpool,
            tc.tile_pool(name="pt", bufs=2, space="PSUM") as ptpool,
            tc.tile_pool(name="pm", bufs=3, space="PSUM") as pmpool,
        ):
            # bias row + weights via Pool/SWDGE before make_identity: the
            # 1-descriptor brow lands ~1.75us in (bias chain starts early),
            # the HWDGE slot frees up so L0 leads SP's queue
            brow = const.tile([1, CM], f32, tag="brow")
            nc.gpsimd.dma_start(out=brow[:], in_=bias[None, :])
            xst = const.tile([P, BS], f32, tag="xst")
            nc.gpsimd.dma_start(out=xst[:], in_=xs[:, :])
            wt = const.tile([P, NG * WN], bf16, tag="wtr")
            nc.gpsimd.dma_start(out=wt[:], in_=bigw[:, :])

            ident0 = const.tile([P, P], f32)
            make_identity(nc, ident0)
            # bf16 identity: ScalarE cast-copy, also puts transposes' dep on
            # a single engine
            ident = const.tile([P, P], bf16, tag="ident2")
            nc.scalar.copy(out=ident[:], in_=ident0[:])
            identr = ident[:, :]

            xins = []

            def load(b):
                # deep rings: loads prefetch far ahead so the DMA engines
                # stay busy while late-pipeline stores trail compute
                x8 = xpool.tile([P, HC], i8, bufs=10)
                nc.sync.dma_start(out=x8[:], in_=xq8[b])
                xb = xpool.tile([P, HC], bf16, tag="xbf", bufs=10)
                nc.sync.dma_start(out=xb[:], in_=xqb[b])
                xins.append((x8, xb))

            browr = const.tile([1, CM], bf16, tag="browr")
            nc.scalar.copy(out=browr[:], in_=brow[:])
            ones0 = const.tile([1, P], f32, tag="ones0")
            nc.gpsimd.memset(ones0, 1.0)
            onesr = const.tile([1, P], bf16, tag="onesr")
            nc.scalar.copy(out=onesr[:], in_=ones0[:])
            # broadcast bias tile for the DVE-add half of the evacs
            bt = const.tile([P, CM], f32)
            for h in range(2):
                pb = ptpool.tile([P, CM // 2], f32, tag="pt")
                nc.tensor.matmul(
                    pb[:],
                    onesr[:],
                    browr[:, h * (CM // 2) : (h + 1) * (CM // 2)],
                    start=True,
                    stop=True,
                )
                nc.scalar.copy(
                    out=bt[:, h * (CM // 2) : (h + 1) * (CM // 2)], in_=pb[:]
                )
            # bias cols (g, c, u) -> view (p, g, c, u)
            btv = bt[:, :].rearrange("p (g c u) -> p g c u", g=NG, c=COUT)

            load(0)
            load(1)
            load(2)
            load(3)
            load(4)
            load(5)
            load(6)
            load(7)

            def transpose_phase(b):
                x8, xb = xins[b]
                # dequant fl 0-1 on Pool: bf16 = int8 * scale[b,fh]
                deq = deqpool.tile([P, HC], bf16)
                nc.gpsimd.tensor_scalar(
                    out=deq[:],
                    in0=x8[:],
                    scalar1=xst[:, b : b + 1],
                    scalar2=None,
                    op0=mybir.AluOpType.mult,
                )
                # ---- transpose: XT_g[(nl*16+i), fl*128 + fh]
                # two groups share one 1-bank bf16 psum tile and drain with
                # one fat copy: halves the transpose<->copy ring ping-pong
                # and the per-copy engine-init overhead
                xt2 = []
                for gp in range(2):
                    pt = ptpool.tile([P, 2 * FL * P], bf16)
                    for gg in range(2):
                        g = 2 * gp + gg
                        for fl in range(FL):
                            src = (
                                deq[:, fl * 512 + g * P : fl * 512 + (g + 1) * P]
                                if fl < 2
                                else xb[:, (fl - 2) * 512 + g * P : (fl - 2) * 512 + (g + 1) * P]
                            )
                            nc.tensor.transpose(
                                pt[:, gg * 512 + fl * P : gg * 512 + (fl + 1) * P],
                                src,
                                identr,
                            )
                    xt = xtpool.tile([P, 2 * FL * P], bf16)
                    # DVE all-bf16 copy runs in 2x mode; split with ACT
                    if gp == 0:
                        nc.vector.tensor_copy(out=xt[:], in_=pt[:])
                    else:
                        nc.scalar.copy(out=xt[:], in_=pt[:])
                    xt2.append(xt)
                xts = [
                    xt2[g // 2][:, (g % 2) * 512 : (g % 2) * 512 + FL * P]
                    for g in range(NG)
                ]
                return xts

            def matmul_store_phase(b, xts):
                # ---- bf16 matmul (+bias row) -> copy evac into STB
                stb = stpool.tile([P, COUT * FL * N * K], bf16)
                stv = stb[:, :].rearrange(
                    "p (c fl m) -> p c fl m", c=COUT, fl=FL
                )
                for fl in range(FL):
                    # one 1024-col psum tile (2 banks) per fl: 4 group
                    # matmuls, then a single fat evac (fewer engine-init
                    # penalties than per-group-pair evacs)
                    pm = pmpool.tile([P, NG * WN], f32)
                    pe_bias = fl % 2 == 0
                    for g in range(NG):
                        nc.tensor.matmul(
                            pm[:, g * WN : (g + 1) * WN],
                            xts[g][:, fl * P : (fl + 1) * P],
                            wt[:, g * WN : (g + 1) * WN],
                            start=True,
                            stop=not pe_bias,
                        )
                        if pe_bias:
                            # bias via K=1 accumulate: psum += ones.T @ brow
                            nc.tensor.matmul(
                                pm[:, g * WN : (g + 1) * WN],
                                onesr[:],
                                browr[:, g * WN : (g + 1) * WN],
                                start=False,
                                stop=True,
                            )
                    # dst cols c*256 + fl*64 + g*U + u
                    dst = stv[:, :, fl, :].rearrange("p c (g u) -> p g c u", g=NG)
                    src = pm[:, :].rearrange("p (g c u) -> p g c u", g=NG, c=COUT)
                    # PE-bias fls evac as ACT copy; the others as DVE-add
                    if pe_bias:
                        nc.scalar.copy(out=dst, in_=src)
                    else:
                        nc.vector.tensor_add(out=dst, in0=src, in1=btv[:, :])

                # ---- store: out[b, c, fh*4+fl, m] <- stb[fh, (c, fl, m)]
                # bf16: full store per b — (fl m) runs are 512B at full
                # descriptor rate; an fl-half split would drop to 256B (2x)
                nc.sync.dma_start(
                    out=out[b].rearrange("c (fh fl) m -> fh c fl m", fl=FL),
                    in_=stb[:, :].rearrange("p (c fl m) -> p c fl m", c=COUT, fl=FL),
                )

            # software pipeline: transposes/xt-copies for b+1 issue BEFORE
            # the matmul+evac phase of b, so PE's in-order queue never
            # starves ACT/DVE of next-batch XT work during b's matmuls
            pend = transpose_phase(0)
            for b in range(BS):
                if b + 8 < BS:
                    load(b + 8)
                if b + 1 < BS:
                    nxt = transpose_phase(b + 1)
                else:
                    nxt = None
                matmul_store_phase(b, pend)
                pend = nxt
    nc.compile()
    return nc


def _fold_weights(W1, b1, W2, b2):
    # Weff[n,i,c,k] = sum_o W1[n,i,o,k] * W2[n,o,c]; beff[n,c] = b1[n]@W2[n] + b2[n]
    Weff = np.einsum("niok,noc->nick", W1, W2).astype(np.float32)
    beff = (np.einsum("no,noc->nc", b1, W2) + b2).astype(np.float32)

    # bigw[nl*CIN + i, g*WN + c*U + 2*nl + k] = Weff[g*GN + nl, i, c, k]
    bigw = np.zeros((GN, CIN, NG, COUT, U), np.float32)  # [nl, i, g, c, u]
    nn, ii, gg, cc, kk = np.meshgrid(
        np.arange(GN), np.arange(CIN), np.arange(NG), np.arange(COUT),
        np.arange(K), indexing="ij",
    )
    w5 = Weff.reshape(NG, GN, CIN, COUT, K)  # [g, nl, i, c, k]
    bigw[nn, ii, gg, cc, 2 * nn + kk] = w5[gg, nn, ii, cc, kk]
    import ml_dtypes

    bigw = np.ascontiguousarray(bigw.reshape(P, NG * WN).astype(ml_dtypes.bfloat16))

    # bias_flat[g*WN + c*U + u] = beff[g*GN + u//K, c]  (matmul rhs layout)
    b4 = beff.reshape(NG, GN, COUT)  # [g, nl, c]
    bias_gcu = np.repeat(b4, K, axis=1)  # [g, u=(nl,k), c]
    bias_flat = np.ascontiguousarray(bias_gcu.transpose(0, 2, 1).reshape(-1))
    return bigw, bias_flat


def kernel(x, W1, b1, W2, b2):
    global _nc_cache
    import ml_dtypes
    from concourse.bass_utils import run_bass_kernel_spmd

    # host pre-repack: xq[b, fh, fl*512+g*128+nl*16+i]; fl 0-1 half as
    # int8 + per-(b,fh) scale, fl 2-3 half as bf16
    x = np.asarray(x, dtype=np.float32)
    xb2 = (
        x.reshape(B, CIN, P, FL, NG, GN)
        .transpose(0, 2, 3, 4, 5, 1)
        .reshape(B, P, CIN * FL * N)
    )
    HC = CIN * FL * N // 2
    lo, hi = xb2[:, :, :HC], xb2[:, :, HC:]
    sc = np.maximum(np.abs(lo).max(axis=2) / 127.0, 1e-30)  # [B, P]
    xq8 = np.clip(np.rint(lo / sc[:, :, None]), -127, 127).astype(np.int8)
    xqb = np.ascontiguousarray(hi.astype(ml_dtypes.bfloat16))
    xq8 = np.ascontiguousarray(xq8)
    bigw, bias_flat = _fold_weights(
        np.asarray(W1, np.float32),
        np.asarray(b1, np.float32),
        np.asarray(W2, np.float32),
        np.asarray(b2, np.float32),
    )
    if _nc_cache is None:
        _nc_cache = _build()
    nc = _nc_cache
    in_maps = [
        {
            "xq8": xq8[d * BS : (d + 1) * BS],
            "xqb": xqb[d * BS : (d + 1) * BS],
            "xs": np.ascontiguousarray(sc[d * BS : (d + 1) * BS].T),
            "bigw": bigw,
            "bias": bias_flat,
        }
        for d in range(NCORES)
    ]
    res = run_bass_kernel_spmd(nc, in_maps, list(range(NCORES)))
    return np.concatenate(
        [res.results[d]["out"].astype(np.float32) for d in range(NCORES)], axis=0
    )
